# revision 36
# baseline (speedup 1.0000x reference)
"""Trainium2 Bass kernel for nn_DiscretisedBNF (discretised BNF loss).

Math: the reference's (B, D, K=128) clamped-CDF bin sum Abel-collapses to

    pO[b,d] = -127/256 - (1/128)*Sigma + (127/256)*erf(z_127),
    Sigma   = sum_{k=1..127} erf(z_k),  z_k = (e_k - mu_x)*inv

and Sigma is a uniform-grid Riemann sum of erf, so by Poisson summation
it equals the midpoint integral up to O(exp(-pi^2/s^2)) (s = inv/64):

    Sigma ~= (1/s)[ z_b*erf(z_b) - z_a*erf(z_a) + (e^{-z_b^2}-e^{-z_a^2})/sqrt(pi) ]
    z_a = inv*(-0.9921875) - mu_x*inv,  z_b = inv*(0.9921875) - mu_x*inv

This replaces the whole 127-bin binning phase (4.2M tanh + z/q matmuls
per core) with ~5 ACT passes and ~15 vector ops per [128,256] tile.
erf is evaluated as tanh((2/sqrt(pi))*(z + c*z^3)) (max abs err 3.6e-4),
so exp+tanh+square+prelu all live in the one resident ACT table set
(exp_and_others) -- no table switches.  End-to-end numpy mirror of the
device math (incl. fp8/bf16 quantization): rel err ~8e-5.

Constant foldings: mm2's ln-tile bias rows add -ln(cexp) (hi/lo bf16
split) so that  inv = exp(-PLN)  and  1/(128 s) = exp(PLN - ln2)  come
straight out of ACT with scalar biases; 1/sqrt(pi) is folded into the
exp bias.

Sharding (8 cores, full inputs in, full output out): mm1 replicated
(fp8 DoubleRow), W2 column-sharded 128+128 cols per core, epilogue
data-parallel on the core's [128 d x 256 b] tile. Output is a single
f32 partial per core (cross-partition reduce via a ones-matmul) so the
final DMA is one 4-byte descriptor. Host sums 8 partials.

PE warm-up: ~8 junk N=512 matmuls on a memset tile right at kernel
start keep HAM from running mm1 at the cold 1.2 GHz clock.
"""

import sys

sys.path.insert(0, "/opt/trn_rl_repo")

import numpy as np
import ml_dtypes

import concourse.bass as bass
import concourse.tile as tile
from concourse import bacc, mybir
from concourse.alu_op_type import AluOpType
from concourse.bass_utils import run_bass_kernel_spmd

B, D, H = 256, 1024, 2048
NCORES = 8
DSL = D // NCORES  # 128 d-columns per core
SIGMA1 = 0.02

F32 = mybir.dt.float32
BF16 = mybir.dt.bfloat16
FP8 = mybir.dt.float8e4
BFNP = ml_dtypes.bfloat16
F8NP = ml_dtypes.float8_e4m3

ERFA = float(2.0 / np.sqrt(np.pi))      # tanh scale
ERFC = float(0.10091075 / ERFA)          # z^3 coefficient (fit, err 3.6e-4)
LNPI2 = float(0.5 * np.log(np.pi))       # folded into exp(-z^2) bias
LN2 = float(np.log(2.0))

# fb blob (fp8, [2, 2, 3072]) offsets in the last dim. All bias matmuls
# run as K=4 fp8 DoubleRow so mm1/mm2 never switch dtype on the PE.
# Contraction rows are (p, r) pairs: (0,0), (1,0), (0,1), (1,1).
FB_TV = 0        # mm1 bias rhs:  (0,0)=t, (1,0)=ones
FB_W1T = 256     # mm1 bias lhsT: (0,0)=W1[D,:], (1,0)=b1
FB_LNW = 2304    # LN bias lhsT:  (0,0)=b2ln, others ones
FB_LNR = 2432    # LN bias rhs:   (0,0)=ones, then -lnCE hi/mid/lo fp8 split
FB_MUW = 2688    # MU bias lhsT:  (0,0)=b2mu
FB_MUR = 2816    # MU bias rhs:   (0,0)=ones
FB_W = 3072

# ep blob (f32, 128 partitions) column offsets
EP_S, EP_RM, EP_XC, EP_SQW, EP_ONE = 0, 256, 512, 768, 1024
EP_BLN2, EP_BPI = 1025, 1026   # bias columns: -ln2, -0.5*ln(pi)
EP_W = 1027


def _build(debug=False):
    nc = bacc.Bacc("TRN2", target_bir_lowering=False, debug=False,
                   num_devices=NCORES)

    d_muT = nc.dram_tensor("muT8", (128, 8 * B), FP8, kind="ExternalInput")
    d_w1 = nc.dram_tensor("w1m", (128, 16 * 8 * 128), FP8,
                          kind="ExternalInput")
    d_w2 = nc.dram_tensor("w2m", (128, 2 * 8 * 2 * 128), FP8,
                          kind="ExternalInput")
    d_fb = nc.dram_tensor("fb", (2, 2 * FB_W), FP8, kind="ExternalInput")
    d_ep = nc.dram_tensor("ep", (128, EP_W), F32, kind="ExternalInput")
    d_out = nc.dram_tensor("outp", (2, 1), F32, kind="ExternalOutput")

    MULT, ADD, SUB, BYP = (AluOpType.mult, AluOpType.add,
                           AluOpType.subtract, AluOpType.bypass)
    AF = mybir.ActivationFunctionType
    DR = mybir.MatmulPerfMode.DoubleRow

    with tile.TileContext(nc) as tc:
        with (
            tc.tile_pool(name="weights", bufs=1) as wpool,
            tc.tile_pool(name="work", bufs=1) as work,
        ):
            muT = wpool.tile([128, 8, B], FP8)
            w1s = [wpool.tile([128, 2, 8, 128], FP8, name=f"w1s{i}")
                   for i in range(8)]
            w2 = wpool.tile([128, 2, 8, 2, 128], FP8)
            fb = wpool.tile([2, 2, FB_W], FP8)
            eps = wpool.tile([128, EP_W], F32)
            jw = wpool.tile([128, 640], BF16)
            hT = work.tile([128, 16, B], FP8)

            with (
                tc.tile_pool(name="psJ", bufs=1,
                             space=bass.MemorySpace.PSUM) as psJ,
                tc.tile_pool(name="psA", bufs=4,
                             space=bass.MemorySpace.PSUM) as psA,
                tc.tile_pool(name="psO", bufs=1,
                             space=bass.MemorySpace.PSUM) as psO,
            ):
                # ---- input DMAs: sync (HWDGE) carries the mm1-critical
                # tensors in need-order. The mm2/epilogue tensors go on
                # gpsimd (SWDGE) but are deferred behind mm1's first tile
                # (dummy dep below) so they don't steal DMA-engine
                # bandwidth from the mm1-critical transfers.
                nc.gpsimd.memset(jw[:], 0.0)
                # mm1-critical transfers split in halves across three
                # otherwise-idle issue queues so tile 0's operands land
                # ~2us earlier; everything else trails on the sync queue.
                nc.sync.dma_start(w1s[0][:, 0], d_w1.ap()[:, 0:1024])
                nc.scalar.dma_start(muT[:, 0:4, :], d_muT.ap()[:, 0:1024])
                nc.sync.dma_start(w1s[0][:, 1], d_w1.ap()[:, 1024:2048])
                nc.scalar.dma_start(muT[:, 4:8, :], d_muT.ap()[:, 1024:2048])
                nc.scalar.dma_start(fb[:], d_fb.ap()[:])
                for s4 in range(1, 8):
                    nc.sync.dma_start(
                        w1s[s4][:], d_w1.ap()[:, s4 * 2048:(s4 + 1) * 2048])
                # mm2/epilogue tensors at the sync-queue tail: their
                # transfers start only after the mm1-critical ones, so
                # they don't steal DMA-engine bandwidth from them.
                nc.sync.dma_start(w2[:], d_w2.ap()[:])
                nc.sync.dma_start(eps[:], d_ep.ap()[:])

                # ---- PE warm-up: dense junk matmuls (no DMA deps). mm1's
                # first tiles may start cold; HAM flips ~3.4us into the
                # combined junk+mm1 dense stream.
                jp = psJ.tile([128, 512], F32)
                for _ in range(5):
                    nc.tensor.matmul(jp[:], jw[:, 0:128], jw[:, 128:640],
                                     start=True, stop=True)

                # ---- mm1: hT[m] = PRelu(W1^T mu_cat^T) fp8 DoubleRow;
                # t-row and b1 folded in as a K=2 bf16 matmul.
                for m in range(16):
                    ph = psA.tile([128, B], F32, tag="ph")
                    for j in range(4):
                        nc.tensor.matmul(
                            ph[:], w1s[m // 2][:, m % 2, 2 * j:2 * j + 2, :],
                            muT[:, 2 * j:2 * j + 2, :],
                            start=(j == 0), stop=False, perf_mode=DR)
                    ms = slice(FB_W1T + m * 128, FB_W1T + (m + 1) * 128)
                    nc.tensor.matmul(ph[:], fb[:, :, ms],
                                     fb[:, :, FB_TV:FB_TV + B],
                                     start=False, stop=True, perf_mode=DR)
                    nc.scalar.activation(hT[:, m, :], ph[:], AF.Prelu,
                                         bias=0.0, scale=1.0, alpha=0.01)

                # ---- mm2: PMU = W2mu^T hT + b2mu first (so the a4/md
                # vector ops overlap the LN matmuls), then PLN = W2ln^T hT
                # + b2ln - lnCE (hi/lo bf16 rows). M=128, fp8 DoubleRow.
                pmu = psO.tile([128, B], F32, name="pmu")
                for j in range(8):
                    nc.tensor.matmul(pmu[:], w2[:, 0, j, :, :],
                                     hT[:, 2 * j:2 * j + 2, :],
                                     start=(j == 0), stop=False, perf_mode=DR)
                nc.tensor.matmul(pmu[:], fb[:, :, FB_MUW:FB_MUW + 128],
                                 fb[:, :, FB_MUR:FB_MUR + B],
                                 start=False, stop=True, perf_mode=DR)
                pln = psO.tile([128, B], F32, name="pln")
                for j in range(8):
                    nc.tensor.matmul(pln[:], w2[:, 1, j, :, :],
                                     hT[:, 2 * j:2 * j + 2, :],
                                     start=(j == 0), stop=False, perf_mode=DR)
                nc.tensor.matmul(pln[:], fb[:, :, FB_LNW:FB_LNW + 128],
                                 fb[:, :, FB_LNR:FB_LNR + B],
                                 start=False, stop=True, perf_mode=DR)

                # ---- epilogue: two column halves pipelined across
                # ACT/DVE/GpSimd. Sign trick: z*erf(z) and e^{-z^2} are
                # even, so we compute -z (saving the mu_x*inv op) and only
                # fix the sign of the standalone erf(z_127) term.
                HB = B // 2
                inv = work.tile([128, B], F32, name="inv")
                nc.scalar.activation(inv[:], pln[:], AF.Exp,
                                     bias=0.0, scale=-1.0)
                isp = work.tile([128, B], F32, name="isp")

                a4 = work.tile([128, B], F32, name="a4")
                md = work.tile([128, B], F32, name="md")
                zab = work.tile([128, 2, 3, HB], F32, name="zab")
                sq = work.tile([128, 2, 3, HB], F32, name="sq")
                cu = work.tile([128, 2, 3, HB], F32, name="cu")
                uu = work.tile([128, 2, 3, HB], F32, name="uu")
                erf = work.tile([128, 2, 3, HB], F32, name="erf")
                exg = work.tile([128, 2, 2, HB], F32, name="exg")
                a1 = work.tile([128, B], F32, name="a1")
                b1t = work.tile([128, B], F32, name="b1t")
                c1 = work.tile([128, B], F32, name="c1")
                d1 = work.tile([128, B], F32, name="d1")
                e1 = work.tile([128, B], F32, name="e1")
                sg = work.tile([128, B], F32, name="sg")
                gg = work.tile([128, B], F32, name="gg")
                hh = work.tile([128, B], F32, name="hh")
                hs = work.tile([128, B], F32, name="hs")
                h2 = work.tile([128, B], F32, name="h2")
                part = work.tile([128, 2], F32, name="part")

                def front_md(h):
                    bs = slice(h * HB, (h + 1) * HB)
                    nc.vector.tensor_tensor(
                        a4[:, bs], eps[:, EP_RM + h * HB:EP_RM + h * HB + HB],
                        pmu[:, bs], MULT)
                    nc.vector.tensor_tensor(
                        md[:, bs], eps[:, EP_S + h * HB:EP_S + h * HB + HB],
                        a4[:, bs], SUB)

                def front(h):
                    bs = slice(h * HB, (h + 1) * HB)
                    # zab holds -z: (-z_a, -z_b, -z_7)
                    for slot, c, op in ((0, 0.9921875, ADD),
                                        (1, 0.9921875, SUB),
                                        (2, 0.984375, SUB)):
                        nc.vector.scalar_tensor_tensor(
                            zab[:, h, slot, :], md[:, bs], c, inv[:, bs],
                            op0=op, op1=MULT)

                def cubic(h):
                    nc.vector.tensor_tensor(cu[:, h], sq[:, h], zab[:, h],
                                            MULT)
                    nc.vector.scalar_tensor_tensor(uu[:, h], cu[:, h], ERFC,
                                                   zab[:, h], op0=MULT,
                                                   op1=ADD)

                def tail(h):
                    bs = slice(h * HB, (h + 1) * HB)
                    nc.vector.tensor_tensor(a1[:, bs], zab[:, h, 1, :],
                                            erf[:, h, 1, :], MULT)
                    # gg = -k*erf7 - XC computes off the critical chain
                    nc.vector.scalar_tensor_tensor(
                        gg[:, bs], erf[:, h, 2, :], -127.0 / 256.0,
                        eps[:, EP_XC + h * HB:EP_XC + h * HB + HB],
                        op0=MULT, op1=SUB)
                    nc.vector.tensor_tensor(d1[:, bs], a1[:, bs], b1t[:, bs],
                                            SUB)
                    nc.vector.tensor_tensor(e1[:, bs], d1[:, bs], c1[:, bs],
                                            ADD)
                    nc.vector.tensor_tensor(sg[:, bs], e1[:, bs], isp[:, bs],
                                            MULT)
                    nc.vector.tensor_tensor(hh[:, bs], gg[:, bs], sg[:, bs],
                                            SUB)
                    nc.vector.tensor_tensor(
                        hs[:, bs], hh[:, bs],
                        eps[:, EP_SQW + h * HB:EP_SQW + h * HB + HB], MULT)
                    nc.vector.scalar_tensor_tensor(
                        h2[:, bs], hs[:, bs], 1.0, hs[:, bs],
                        op0=BYP, op1=MULT, accum_out=part[:, h:h + 1])

                def gp_side(h):
                    bs = slice(h * HB, (h + 1) * HB)
                    nc.gpsimd.tensor_tensor(b1t[:, bs], zab[:, h, 0, :],
                                            erf[:, h, 0, :], MULT)
                    nc.gpsimd.tensor_tensor(c1[:, bs], exg[:, h, 1, :],
                                            exg[:, h, 0, :], SUB)

                front_md(0)
                front_md(1)
                front(0)
                front(1)
                nc.scalar.activation(sq[:, 0], zab[:, 0], AF.Square,
                                     bias=0.0, scale=1.0)
                cubic(0)
                nc.scalar.activation(erf[:, 0], uu[:, 0], AF.Tanh,
                                     bias=0.0, scale=ERFA)
                nc.scalar.activation(sq[:, 1], zab[:, 1], AF.Square,
                                     bias=0.0, scale=1.0)
                nc.scalar.activation(exg[:, 0], sq[:, 0, 0:2, :], AF.Exp,
                                     bias=eps[:, EP_BPI:EP_BPI + 1],
                                     scale=-1.0)
                cubic(1)
                nc.scalar.activation(erf[:, 1], uu[:, 1], AF.Tanh,
                                     bias=0.0, scale=ERFA)
                nc.scalar.activation(exg[:, 1], sq[:, 1, 0:2, :], AF.Exp,
                                     bias=eps[:, EP_BPI:EP_BPI + 1],
                                     scale=-1.0)
                # isp is only consumed by the late sg step; emit it after
                # the ladder so it doesn't delay SQ/TANH on the ACT queue.
                nc.scalar.activation(isp[:], pln[:], AF.Exp,
                                     bias=eps[:, EP_BLN2:EP_BLN2 + 1],
                                     scale=1.0)
                gp_side(0)
                tail(0)
                gp_side(1)
                tail(1)

                # cross-partition reduce: [2,1] psum via ones-matmul, so
                # the output DMA is two 4-byte descriptors.
                ps1 = psO.tile([2, 1], F32, name="ps1")
                nc.tensor.matmul(ps1[:], part[:],
                                 eps[:, EP_ONE:EP_ONE + 1],
                                 start=True, stop=True)
                sres = work.tile([2, 1], F32, name="sres")
                nc.vector.tensor_copy(sres[:], ps1[:])
                nc.sync.dma_start(d_out.ap()[:], sres[:])

    nc.compile()
    return nc


def host_prep(x, t, noise, W1, b1, W2, b2):
    """Build the per-core in_maps (host-side packing + tiny per-row math)."""
    f32 = np.float32
    tv = t[:, 0].astype(f32)
    assert float(tv.min()) > 1e-8, "low-t mask path not implemented"
    gamma = (1.0 - np.power(f32(SIGMA1), f32(2.0) * tv)).astype(f32)
    assert float(gamma.min()) > 0.0
    r = np.sqrt((1.0 - gamma) / gamma).astype(f32)
    lnce = np.log(1.0 / (r * np.sqrt(f32(2.0)))).astype(f32)
    v = (-lnce).astype(f32)
    nlh = v.astype(F8NP)
    nlm = (v - nlh.astype(f32)).astype(F8NP)
    nll = (v - nlh.astype(f32) - nlm.astype(f32)).astype(F8NP)
    sqw = np.power(f32(SIGMA1), -tv).astype(f32)

    mu = (gamma[:, None] * x + (gamma * (1 - gamma))[:, None] * noise
          ).astype(f32)
    muT8 = np.ascontiguousarray(
        mu.T.reshape(8, 128, B).transpose(1, 0, 2).reshape(128, 8 * B)
        .astype(F8NP))

    # w1m[p, (m*8+k)*128 + c] = W1[k*128+p, m*128+c]
    w1f = W1[:D].astype(f32).reshape(8, 128, 16, 128)
    w1m = np.ascontiguousarray(
        w1f.transpose(1, 2, 0, 3).reshape(128, 16 * 8 * 128).astype(F8NP))

    fbv = np.zeros((2, 2, FB_W), dtype=F8NP)
    fbv[0, 0, FB_TV:FB_TV + B] = tv.astype(F8NP)
    fbv[1, 0, FB_TV:FB_TV + B] = F8NP(1.0)
    fbv[0, 0, FB_W1T:FB_W1T + H] = W1[D].astype(F8NP)
    fbv[1, 0, FB_W1T:FB_W1T + H] = b1.astype(F8NP)
    fbv[1, 0, FB_LNW:FB_LNW + 128] = F8NP(1.0)
    fbv[0, 1, FB_LNW:FB_LNW + 128] = F8NP(1.0)
    fbv[1, 1, FB_LNW:FB_LNW + 128] = F8NP(1.0)
    fbv[0, 0, FB_LNR:FB_LNR + B] = F8NP(1.0)
    fbv[1, 0, FB_LNR:FB_LNR + B] = nlh
    fbv[0, 1, FB_LNR:FB_LNR + B] = nlm
    fbv[1, 1, FB_LNR:FB_LNR + B] = nll
    fbv[0, 0, FB_MUR:FB_MUR + B] = F8NP(1.0)

    epv = np.zeros((128, EP_W), dtype=f32)
    epv[:, EP_RM:EP_RM + B] = r[None, :]
    epv[:, EP_SQW:EP_SQW + B] = sqw[None, :]
    epv[:, EP_ONE] = 1.0
    epv[:, EP_BLN2] = -LN2
    epv[:, EP_BPI] = -LNPI2

    S_full = (x + (1.0 - gamma)[:, None] * noise).astype(f32)
    XC_full = (x + f32(127.0 / 256.0)).astype(f32)

    in_maps = []
    for i in range(NCORES):
        dsl = slice(i * DSL, (i + 1) * DSL)
        # w2m[p, ((half*8+j)*2+rr)*128+c] = W2[(2j+rr)*128+p, half*D + col]
        w2c = np.stack([W2[:, dsl], W2[:, D + i * DSL:D + (i + 1) * DSL]],
                       axis=0)  # [2, 2048, 128]
        w2m = np.ascontiguousarray(
            w2c.reshape(2, 16, 128, 128).transpose(2, 0, 1, 3)
            .reshape(128, 2 * 16 * 128).astype(F8NP))
        fbi = fbv.copy()
        fbi[0, 0, FB_LNW:FB_LNW + 128] = b2[D + i * DSL:D + (i + 1) * DSL
                                            ].astype(F8NP)
        fbi[0, 0, FB_MUW:FB_MUW + 128] = b2[dsl].astype(F8NP)
        epi = epv.copy()
        epi[:, EP_S:EP_S + B] = S_full[:, dsl].T
        epi[:, EP_XC:EP_XC + B] = XC_full[:, dsl].T
        in_maps.append({
            "muT8": muT8, "w1m": w1m, "w2m": w2m,
            "fb": fbi.reshape(2, 2 * FB_W), "ep": epi,
        })
    return in_maps


_nc_cache = {}


def get_nc(debug=False):
    if debug not in _nc_cache:
        _nc_cache[debug] = _build(debug)
    return _nc_cache[debug]


def run_on_cores(inputs, trace=False, debug=False, tmpdir=None):
    nc = get_nc(debug)
    in_maps = host_prep(**inputs)
    res = run_bass_kernel_spmd(nc, in_maps, core_ids=list(range(NCORES)),
                               trace=trace, tmpdir=tmpdir)
    total = np.float64(0.0)
    for i in range(NCORES):
        total += res.results[i]["outp"].astype(np.float64).sum()
    loss = np.float32(-np.log(np.float32(SIGMA1)) * total / float(B * D))
    return loss, res


_reset_done = [False]


def _maybe_reset_device():
    # Clear a wedged NRT exec unit left by a previous process. Best-effort.
    if _reset_done[0]:
        return
    _reset_done[0] = True
    try:
        import os
        import ctypes
        so = "/opt/axon/libaxon_pjrt.so"
        if os.path.exists(so):
            import jax

            jax.devices()
            lib = ctypes.CDLL(so)
            lib.axon_reset.restype = ctypes.c_int64
            lib.axon_reset()
    except Exception:
        pass


def kernel(**inputs):
    _maybe_reset_device()
    inputs = {k: np.asarray(v) for k, v in inputs.items()}
    loss, _ = run_on_cores(inputs)
    return np.asarray(loss, dtype=np.float32)


# revision 37
# speedup vs baseline: 1.0288x; 1.0288x over previous
"""Trainium2 Bass kernel for nn_DiscretisedBNF (discretised BNF loss).

Math: the reference's (B, D, K=128) clamped-CDF bin sum Abel-collapses to

    pO[b,d] = -127/256 - (1/128)*Sigma + (127/256)*erf(z_127),
    Sigma   = sum_{k=1..127} erf(z_k),  z_k = (e_k - mu_x)*inv

and Sigma is a uniform-grid Riemann sum of erf, so by Poisson summation
it equals the midpoint integral up to O(exp(-pi^2/s^2)) (s = inv/64):

    Sigma ~= (1/s)[ z_b*erf(z_b) - z_a*erf(z_a) + (e^{-z_b^2}-e^{-z_a^2})/sqrt(pi) ]
    z_a = inv*(-0.9921875) - mu_x*inv,  z_b = inv*(0.9921875) - mu_x*inv

This replaces the whole 127-bin binning phase (4.2M tanh + z/q matmuls
per core) with ~5 ACT passes and ~15 vector ops per [128,256] tile.
erf is evaluated as tanh((2/sqrt(pi))*(z + c*z^3)) (max abs err 3.6e-4),
so exp+tanh+square+prelu all live in the one resident ACT table set
(exp_and_others) -- no table switches.  End-to-end numpy mirror of the
device math (incl. fp8/bf16 quantization): rel err ~8e-5.

Constant foldings: mm2's ln-tile bias rows add -ln(cexp) (hi/lo bf16
split) so that  inv = exp(-PLN)  and  1/(128 s) = exp(PLN - ln2)  come
straight out of ACT with scalar biases; 1/sqrt(pi) is folded into the
exp bias.

Sharding (8 cores, full inputs in, full output out): mm1 replicated
(fp8 DoubleRow), W2 column-sharded 128+128 cols per core, epilogue
data-parallel on the core's [128 d x 256 b] tile. Output is a single
f32 partial per core (cross-partition reduce via a ones-matmul) so the
final DMA is one 4-byte descriptor. Host sums 8 partials.

PE warm-up: ~8 junk N=512 matmuls on a memset tile right at kernel
start keep HAM from running mm1 at the cold 1.2 GHz clock.
"""

import sys

sys.path.insert(0, "/opt/trn_rl_repo")

import numpy as np
import ml_dtypes

import concourse.bass as bass
import concourse.tile as tile
from concourse import bacc, mybir
from concourse.alu_op_type import AluOpType
from concourse.bass_utils import run_bass_kernel_spmd

B, D, H = 256, 1024, 2048
NCORES = 8
DSL = D // NCORES  # 128 d-columns per core
SIGMA1 = 0.02

F32 = mybir.dt.float32
BF16 = mybir.dt.bfloat16
FP8 = mybir.dt.float8e4
BFNP = ml_dtypes.bfloat16
F8NP = ml_dtypes.float8_e4m3

ERFA = float(2.0 / np.sqrt(np.pi))      # tanh scale
ERFC = float(0.10091075 / ERFA)          # z^3 coefficient (fit, err 3.6e-4)
LNPI2 = float(0.5 * np.log(np.pi))       # folded into exp(-z^2) bias
LN2 = float(np.log(2.0))

# fb blob (fp8, [2, 2, 3072]) offsets in the last dim. All bias matmuls
# run as K=4 fp8 DoubleRow so mm1/mm2 never switch dtype on the PE.
# Contraction rows are (p, r) pairs: (0,0), (1,0), (0,1), (1,1).
FB_TV = 0        # mm1 bias rhs:  (0,0)=t, (1,0)=ones
FB_W1T = 256     # mm1 bias lhsT: (0,0)=W1[D,:], (1,0)=b1
FB_LNW = 2304    # LN bias lhsT:  (0,0)=b2ln, others ones
FB_LNR = 2432    # LN bias rhs:   (0,0)=ones, then -lnCE hi/mid/lo fp8 split
FB_MUW = 2688    # MU bias lhsT:  (0,0)=b2mu
FB_MUR = 2816    # MU bias rhs:   (0,0)=ones
FB_W = 3072

# ep blob (f32, 128 partitions) column offsets
EP_S, EP_RM, EP_XC, EP_SQW, EP_ONE = 0, 256, 512, 768, 1024
EP_BLN2, EP_BPI = 1025, 1026   # bias columns: -ln2, -0.5*ln(pi)
EP_W = 1027


def _build(debug=False):
    nc = bacc.Bacc("TRN2", target_bir_lowering=False, debug=False,
                   num_devices=NCORES)

    d_muT = nc.dram_tensor("muT8", (128, 8 * B), FP8, kind="ExternalInput")
    d_w1 = nc.dram_tensor("w1m", (128, 16 * 8 * 128), FP8,
                          kind="ExternalInput")
    d_w2 = nc.dram_tensor("w2m", (128, 2 * 8 * 2 * 128), FP8,
                          kind="ExternalInput")
    d_fb = nc.dram_tensor("fb", (2, 2 * FB_W), FP8, kind="ExternalInput")
    d_ep = nc.dram_tensor("ep", (128, EP_W), F32, kind="ExternalInput")
    d_out = nc.dram_tensor("outp", (2, 1), F32, kind="ExternalOutput")

    MULT, ADD, SUB, BYP = (AluOpType.mult, AluOpType.add,
                           AluOpType.subtract, AluOpType.bypass)
    AF = mybir.ActivationFunctionType
    DR = mybir.MatmulPerfMode.DoubleRow

    with tile.TileContext(nc) as tc:
        with (
            tc.tile_pool(name="weights", bufs=1) as wpool,
            tc.tile_pool(name="work", bufs=1) as work,
        ):
            muT = wpool.tile([128, 8, B], FP8)
            w1s = [wpool.tile([128, 2, 8, 128], FP8, name=f"w1s{i}")
                   for i in range(8)]
            w2 = wpool.tile([128, 2, 8, 2, 128], FP8)
            fb = wpool.tile([2, 2, FB_W], FP8)
            eps = wpool.tile([128, EP_W], F32)
            jw = wpool.tile([128, 640], BF16)
            hT = work.tile([128, 16, B], FP8)

            with (
                tc.tile_pool(name="psJ", bufs=1,
                             space=bass.MemorySpace.PSUM) as psJ,
                tc.tile_pool(name="psA", bufs=4,
                             space=bass.MemorySpace.PSUM) as psA,
                tc.tile_pool(name="psO", bufs=1,
                             space=bass.MemorySpace.PSUM) as psO,
            ):
                # ---- input DMAs: sync (HWDGE) carries the mm1-critical
                # tensors in need-order. The mm2/epilogue tensors go on
                # gpsimd (SWDGE) but are deferred behind mm1's first tile
                # (dummy dep below) so they don't steal DMA-engine
                # bandwidth from the mm1-critical transfers.
                nc.gpsimd.memset(jw[:], 0.0)
                # mm1-critical transfers split in halves across three
                # otherwise-idle issue queues so tile 0's operands land
                # ~2us earlier; everything else trails on the sync queue.
                nc.sync.dma_start(w1s[0][:, 0], d_w1.ap()[:, 0:1024])
                nc.sync.dma_start(muT[:, 0:4, :], d_muT.ap()[:, 0:1024])
                nc.sync.dma_start(w1s[0][:, 1], d_w1.ap()[:, 1024:2048])
                nc.sync.dma_start(muT[:, 4:8, :], d_muT.ap()[:, 1024:2048])
                nc.scalar.dma_start(fb[:], d_fb.ap()[:])
                for s4 in range(1, 8):
                    nc.sync.dma_start(
                        w1s[s4][:], d_w1.ap()[:, s4 * 2048:(s4 + 1) * 2048])
                # mm2/epilogue tensors at the sync-queue tail: their
                # transfers start only after the mm1-critical ones, so
                # they don't steal DMA-engine bandwidth from them.
                nc.sync.dma_start(w2[:], d_w2.ap()[:])
                nc.sync.dma_start(eps[:], d_ep.ap()[:])

                # ---- PE warm-up: dense junk matmuls (no DMA deps). mm1's
                # first tiles may start cold; HAM flips ~3.4us into the
                # combined junk+mm1 dense stream.
                jp = psJ.tile([128, 512], F32)
                for _ in range(5):
                    nc.tensor.matmul(jp[:], jw[:, 0:128], jw[:, 128:640],
                                     start=True, stop=True)

                # ---- mm1: hT[m] = PRelu(W1^T mu_cat^T) fp8 DoubleRow;
                # t-row and b1 folded in as a K=2 bf16 matmul.
                for m in range(16):
                    ph = psA.tile([128, B], F32, tag="ph")
                    for j in range(4):
                        nc.tensor.matmul(
                            ph[:], w1s[m // 2][:, m % 2, 2 * j:2 * j + 2, :],
                            muT[:, 2 * j:2 * j + 2, :],
                            start=(j == 0), stop=False, perf_mode=DR)
                    ms = slice(FB_W1T + m * 128, FB_W1T + (m + 1) * 128)
                    nc.tensor.matmul(ph[:], fb[:, :, ms],
                                     fb[:, :, FB_TV:FB_TV + B],
                                     start=False, stop=True, perf_mode=DR)
                    nc.scalar.activation(hT[:, m, :], ph[:], AF.Prelu,
                                         bias=0.0, scale=1.0, alpha=0.01)

                # ---- mm2: PMU = W2mu^T hT + b2mu first (so the a4/md
                # vector ops overlap the LN matmuls), then PLN = W2ln^T hT
                # + b2ln - lnCE (hi/lo bf16 rows). M=128, fp8 DoubleRow.
                pmu = psO.tile([128, B], F32, name="pmu")
                for j in range(8):
                    nc.tensor.matmul(pmu[:], w2[:, 0, j, :, :],
                                     hT[:, 2 * j:2 * j + 2, :],
                                     start=(j == 0), stop=False, perf_mode=DR)
                nc.tensor.matmul(pmu[:], fb[:, :, FB_MUW:FB_MUW + 128],
                                 fb[:, :, FB_MUR:FB_MUR + B],
                                 start=False, stop=True, perf_mode=DR)
                pln = psO.tile([128, B], F32, name="pln")
                for j in range(8):
                    nc.tensor.matmul(pln[:], w2[:, 1, j, :, :],
                                     hT[:, 2 * j:2 * j + 2, :],
                                     start=(j == 0), stop=False, perf_mode=DR)
                nc.tensor.matmul(pln[:], fb[:, :, FB_LNW:FB_LNW + 128],
                                 fb[:, :, FB_LNR:FB_LNR + B],
                                 start=False, stop=True, perf_mode=DR)

                # ---- epilogue: two column halves pipelined across
                # ACT/DVE/GpSimd. Sign trick: z*erf(z) and e^{-z^2} are
                # even, so we compute -z (saving the mu_x*inv op) and only
                # fix the sign of the standalone erf(z_127) term.
                HB = B // 2
                inv = work.tile([128, B], F32, name="inv")
                nc.scalar.activation(inv[:], pln[:], AF.Exp,
                                     bias=0.0, scale=-1.0)
                isp = work.tile([128, B], F32, name="isp")

                a4 = work.tile([128, B], F32, name="a4")
                md = work.tile([128, B], F32, name="md")
                zab = work.tile([128, 2, 3, HB], F32, name="zab")
                sq = work.tile([128, 2, 3, HB], F32, name="sq")
                cu = work.tile([128, 2, 3, HB], F32, name="cu")
                uu = work.tile([128, 2, 3, HB], F32, name="uu")
                erf = work.tile([128, 2, 3, HB], F32, name="erf")
                exg = work.tile([128, 2, 2, HB], F32, name="exg")
                a1 = work.tile([128, B], F32, name="a1")
                b1t = work.tile([128, B], F32, name="b1t")
                c1 = work.tile([128, B], F32, name="c1")
                d1 = work.tile([128, B], F32, name="d1")
                e1 = work.tile([128, B], F32, name="e1")
                sg = work.tile([128, B], F32, name="sg")
                gg = work.tile([128, B], F32, name="gg")
                hh = work.tile([128, B], F32, name="hh")
                hs = work.tile([128, B], F32, name="hs")
                h2 = work.tile([128, B], F32, name="h2")
                part = work.tile([128, 2], F32, name="part")

                def front_md(h):
                    bs = slice(h * HB, (h + 1) * HB)
                    nc.vector.tensor_tensor(
                        a4[:, bs], eps[:, EP_RM + h * HB:EP_RM + h * HB + HB],
                        pmu[:, bs], MULT)
                    nc.vector.tensor_tensor(
                        md[:, bs], eps[:, EP_S + h * HB:EP_S + h * HB + HB],
                        a4[:, bs], SUB)

                def front(h):
                    bs = slice(h * HB, (h + 1) * HB)
                    # zab holds -z: (-z_a, -z_b, -z_7)
                    for slot, c, op in ((0, 0.9921875, ADD),
                                        (1, 0.9921875, SUB),
                                        (2, 0.984375, SUB)):
                        nc.vector.scalar_tensor_tensor(
                            zab[:, h, slot, :], md[:, bs], c, inv[:, bs],
                            op0=op, op1=MULT)

                def cubic(h):
                    nc.vector.tensor_tensor(cu[:, h], sq[:, h], zab[:, h],
                                            MULT)
                    nc.vector.scalar_tensor_tensor(uu[:, h], cu[:, h], ERFC,
                                                   zab[:, h], op0=MULT,
                                                   op1=ADD)

                def tail(h):
                    bs = slice(h * HB, (h + 1) * HB)
                    nc.vector.tensor_tensor(a1[:, bs], zab[:, h, 1, :],
                                            erf[:, h, 1, :], MULT)
                    # gg = -k*erf7 - XC computes off the critical chain
                    nc.vector.scalar_tensor_tensor(
                        gg[:, bs], erf[:, h, 2, :], -127.0 / 256.0,
                        eps[:, EP_XC + h * HB:EP_XC + h * HB + HB],
                        op0=MULT, op1=SUB)
                    nc.vector.tensor_tensor(d1[:, bs], a1[:, bs], b1t[:, bs],
                                            SUB)
                    nc.vector.tensor_tensor(e1[:, bs], d1[:, bs], c1[:, bs],
                                            ADD)
                    nc.vector.tensor_tensor(sg[:, bs], e1[:, bs], isp[:, bs],
                                            MULT)
                    nc.vector.tensor_tensor(hh[:, bs], gg[:, bs], sg[:, bs],
                                            SUB)
                    nc.vector.tensor_tensor(
                        hs[:, bs], hh[:, bs],
                        eps[:, EP_SQW + h * HB:EP_SQW + h * HB + HB], MULT)
                    nc.vector.scalar_tensor_tensor(
                        h2[:, bs], hs[:, bs], 1.0, hs[:, bs],
                        op0=BYP, op1=MULT, accum_out=part[:, h:h + 1])

                def gp_side(h):
                    bs = slice(h * HB, (h + 1) * HB)
                    nc.gpsimd.tensor_tensor(b1t[:, bs], zab[:, h, 0, :],
                                            erf[:, h, 0, :], MULT)
                    nc.gpsimd.tensor_tensor(c1[:, bs], exg[:, h, 1, :],
                                            exg[:, h, 0, :], SUB)

                front_md(0)
                front_md(1)
                front(0)
                front(1)
                nc.scalar.activation(sq[:, 0], zab[:, 0], AF.Square,
                                     bias=0.0, scale=1.0)
                cubic(0)
                nc.scalar.activation(erf[:, 0], uu[:, 0], AF.Tanh,
                                     bias=0.0, scale=ERFA)
                nc.scalar.activation(sq[:, 1], zab[:, 1], AF.Square,
                                     bias=0.0, scale=1.0)
                nc.scalar.activation(exg[:, 0], sq[:, 0, 0:2, :], AF.Exp,
                                     bias=eps[:, EP_BPI:EP_BPI + 1],
                                     scale=-1.0)
                cubic(1)
                nc.scalar.activation(erf[:, 1], uu[:, 1], AF.Tanh,
                                     bias=0.0, scale=ERFA)
                nc.scalar.activation(exg[:, 1], sq[:, 1, 0:2, :], AF.Exp,
                                     bias=eps[:, EP_BPI:EP_BPI + 1],
                                     scale=-1.0)
                # isp is only consumed by the late sg step; emit it after
                # the ladder so it doesn't delay SQ/TANH on the ACT queue.
                nc.scalar.activation(isp[:], pln[:], AF.Exp,
                                     bias=eps[:, EP_BLN2:EP_BLN2 + 1],
                                     scale=1.0)
                gp_side(0)
                tail(0)
                gp_side(1)
                tail(1)

                # cross-partition reduce: [2,1] psum via ones-matmul, so
                # the output DMA is two 4-byte descriptors.
                ps1 = psO.tile([2, 1], F32, name="ps1")
                nc.tensor.matmul(ps1[:], part[:],
                                 eps[:, EP_ONE:EP_ONE + 1],
                                 start=True, stop=True)
                sres = work.tile([2, 1], F32, name="sres")
                nc.vector.tensor_copy(sres[:], ps1[:])
                nc.sync.dma_start(d_out.ap()[:], sres[:])

    nc.compile()
    return nc


def host_prep(x, t, noise, W1, b1, W2, b2):
    """Build the per-core in_maps (host-side packing + tiny per-row math)."""
    f32 = np.float32
    tv = t[:, 0].astype(f32)
    assert float(tv.min()) > 1e-8, "low-t mask path not implemented"
    gamma = (1.0 - np.power(f32(SIGMA1), f32(2.0) * tv)).astype(f32)
    assert float(gamma.min()) > 0.0
    r = np.sqrt((1.0 - gamma) / gamma).astype(f32)
    lnce = np.log(1.0 / (r * np.sqrt(f32(2.0)))).astype(f32)
    v = (-lnce).astype(f32)
    nlh = v.astype(F8NP)
    nlm = (v - nlh.astype(f32)).astype(F8NP)
    nll = (v - nlh.astype(f32) - nlm.astype(f32)).astype(F8NP)
    sqw = np.power(f32(SIGMA1), -tv).astype(f32)

    mu = (gamma[:, None] * x + (gamma * (1 - gamma))[:, None] * noise
          ).astype(f32)
    muT8 = np.ascontiguousarray(
        mu.T.reshape(8, 128, B).transpose(1, 0, 2).reshape(128, 8 * B)
        .astype(F8NP))

    # w1m[p, (m*8+k)*128 + c] = W1[k*128+p, m*128+c]
    w1f = W1[:D].astype(f32).reshape(8, 128, 16, 128)
    w1m = np.ascontiguousarray(
        w1f.transpose(1, 2, 0, 3).reshape(128, 16 * 8 * 128).astype(F8NP))

    fbv = np.zeros((2, 2, FB_W), dtype=F8NP)
    fbv[0, 0, FB_TV:FB_TV + B] = tv.astype(F8NP)
    fbv[1, 0, FB_TV:FB_TV + B] = F8NP(1.0)
    fbv[0, 0, FB_W1T:FB_W1T + H] = W1[D].astype(F8NP)
    fbv[1, 0, FB_W1T:FB_W1T + H] = b1.astype(F8NP)
    fbv[1, 0, FB_LNW:FB_LNW + 128] = F8NP(1.0)
    fbv[0, 1, FB_LNW:FB_LNW + 128] = F8NP(1.0)
    fbv[1, 1, FB_LNW:FB_LNW + 128] = F8NP(1.0)
    fbv[0, 0, FB_LNR:FB_LNR + B] = F8NP(1.0)
    fbv[1, 0, FB_LNR:FB_LNR + B] = nlh
    fbv[0, 1, FB_LNR:FB_LNR + B] = nlm
    fbv[1, 1, FB_LNR:FB_LNR + B] = nll
    fbv[0, 0, FB_MUR:FB_MUR + B] = F8NP(1.0)

    epv = np.zeros((128, EP_W), dtype=f32)
    epv[:, EP_RM:EP_RM + B] = r[None, :]
    epv[:, EP_SQW:EP_SQW + B] = sqw[None, :]
    epv[:, EP_ONE] = 1.0
    epv[:, EP_BLN2] = -LN2
    epv[:, EP_BPI] = -LNPI2

    S_full = (x + (1.0 - gamma)[:, None] * noise).astype(f32)
    XC_full = (x + f32(127.0 / 256.0)).astype(f32)

    in_maps = []
    for i in range(NCORES):
        dsl = slice(i * DSL, (i + 1) * DSL)
        # w2m[p, ((half*8+j)*2+rr)*128+c] = W2[(2j+rr)*128+p, half*D + col]
        w2c = np.stack([W2[:, dsl], W2[:, D + i * DSL:D + (i + 1) * DSL]],
                       axis=0)  # [2, 2048, 128]
        w2m = np.ascontiguousarray(
            w2c.reshape(2, 16, 128, 128).transpose(2, 0, 1, 3)
            .reshape(128, 2 * 16 * 128).astype(F8NP))
        fbi = fbv.copy()
        fbi[0, 0, FB_LNW:FB_LNW + 128] = b2[D + i * DSL:D + (i + 1) * DSL
                                            ].astype(F8NP)
        fbi[0, 0, FB_MUW:FB_MUW + 128] = b2[dsl].astype(F8NP)
        epi = epv.copy()
        epi[:, EP_S:EP_S + B] = S_full[:, dsl].T
        epi[:, EP_XC:EP_XC + B] = XC_full[:, dsl].T
        in_maps.append({
            "muT8": muT8, "w1m": w1m, "w2m": w2m,
            "fb": fbi.reshape(2, 2 * FB_W), "ep": epi,
        })
    return in_maps


_nc_cache = {}


def get_nc(debug=False):
    if debug not in _nc_cache:
        _nc_cache[debug] = _build(debug)
    return _nc_cache[debug]


def run_on_cores(inputs, trace=False, debug=False, tmpdir=None):
    nc = get_nc(debug)
    in_maps = host_prep(**inputs)
    res = run_bass_kernel_spmd(nc, in_maps, core_ids=list(range(NCORES)),
                               trace=trace, tmpdir=tmpdir)
    total = np.float64(0.0)
    for i in range(NCORES):
        total += res.results[i]["outp"].astype(np.float64).sum()
    loss = np.float32(-np.log(np.float32(SIGMA1)) * total / float(B * D))
    return loss, res


_reset_done = [False]


def _maybe_reset_device():
    # Clear a wedged NRT exec unit left by a previous process. Best-effort.
    if _reset_done[0]:
        return
    _reset_done[0] = True
    try:
        import os
        import ctypes
        so = "/opt/axon/libaxon_pjrt.so"
        if os.path.exists(so):
            import jax

            jax.devices()
            lib = ctypes.CDLL(so)
            lib.axon_reset.restype = ctypes.c_int64
            lib.axon_reset()
    except Exception:
        pass


def kernel(**inputs):
    _maybe_reset_device()
    inputs = {k: np.asarray(v) for k, v in inputs.items()}
    loss, _ = run_on_cores(inputs)
    return np.asarray(loss, dtype=np.float32)
